# revision 38
# baseline (speedup 1.0000x reference)
"""Trainium2 Bass kernel for nn_ATHP_26388279066955 (sparse_attention / ATHP).

Strategy
--------
8 cores = (batch b in 0..3) x (sequence half in 0..1).  Each core:
  part 1  cumulative-softmax attention via the cancellation
          out5[p] = cumsum(exp(A) * v)[p] / cumsum(exp(A))[p]
          (the cummax in the reference cancels exactly in the ratio),
          computed TRANSPOSED (embT[d_in, t]) with lower-triangular matmuls
          + a carry chain over 128-blocks.  Cores owning the second half
          run 6 "prefix" blocks first (carry only); first-half cores get
          A=-60 junk there so exp()=0 keeps the carry at 0 (uniform SPMD).
  part 2  prologue: W_start/W_conv/W_dec matmuls on embT, GELU via
          exp+reciprocal (avoids the gelu act-table), softplus(10x)/10 via
          relu(u)+ln(1+exp(-|u|)) -> all inside the natural_log_exp set.
  part 3  Monte-Carlo loop over 101 "s-slots" (100 MC samples + 1 slot for
          the dt-endpoint cell_t): tiles are [128 d, G*128 t] so the
          k-contraction matmul needs NO transposes.  tau broadcasts happen
          on the PE (ones[1,128] x ntau-row matmuls into PSUM).
          exp/tanh on ACT, mul on DVE, add on GPSIMD; all bf16.
  part 4  softplus(z)=ln(1+exp(z)) on the packed z-buffer (one act-table
          switch), sum over k via ones-matmul, MC mean, dt weighting,
          log-likelihood term, -> per-core [1,2] partial output.
Host sums the two half partial outputs per batch (the "final all-reduce").
"""

import math
import os
import sys
from contextlib import ExitStack

import numpy as np

sys.path.insert(0, "/opt/trn_rl_repo")

import ml_dtypes  # noqa: E402

B, P, M, DPHI, DIN, K, S = 4, 1536, 4, 32, 128, 20, 100
T = P - 1          # 1535
H = P // 2         # 768 rows per core
NBLK = H // 128    # 6 active blocks per core
NS = S + 1         # 101 s-slots (slot 100 = dt endpoint for cell_t)
GELU_C = math.sqrt(2.0 / math.pi)

_CACHE = {}


def _build_nc():
    import concourse.bass as bass
    import concourse.tile as tile
    from concourse import bacc, mybir

    dt = mybir.dt
    f32, bf16 = dt.float32, dt.bfloat16
    AF = mybir.ActivationFunctionType
    Alu = mybir.AluOpType
    Axis = mybir.AxisListType

    # Make the act-table-load pass resolve Ln to natural_log_exp_and_others
    # (which also holds exp) instead of the exp-less natural_log set, so the
    # softplus Exp/Ln pairs don't thrash table loads.  Only capabilities are
    # REMOVED from the pass's view; emitted set ids stay canonical.
    if not getattr(bacc, "_athp_tables_patched", False):
        _orig_gat = bacc.get_activation_tables

        def _gat(arch):
            t = dict(_orig_gat(arch))
            if "natural_log" in t and "natural_log_exp_and_others" in t:
                t["natural_log"] = set()
            return t

        bacc.get_activation_tables = _gat
        bacc._athp_tables_patched = True

    nc = bacc.Bacc(
        "TRN2",
        target_bir_lowering=False,
        debug=False,
        enable_asserts=False,
        num_devices=8,
    )

    # ---- DRAM I/O ----
    A_all = nc.dram_tensor("A_all", [P, M], f32, kind="ExternalInput").ap()
    V_all = nc.dram_tensor("V_all", [P, DPHI], f32, kind="ExternalInput").ap()
    Ltri = nc.dram_tensor("Ltri", [128, 128], f32, kind="ExternalInput").ap()
    SELm = nc.dram_tensor("SELm", [M, 128], f32, kind="ExternalInput").ap()
    ones1 = nc.dram_tensor("ones1", [1, 128], f32, kind="ExternalInput").ap()
    ones84 = nc.dram_tensor("ones84", [84, 1], bf16, kind="ExternalInput").ap()
    ones20 = nc.dram_tensor("ones20", [20, 1], f32, kind="ExternalInput").ap()
    one11 = nc.dram_tensor("one11", [1, 1], f32, kind="ExternalInput").ap()
    Wst = nc.dram_tensor("Wst", [DIN, DIN], f32, kind="ExternalInput").ap()
    Wcv = nc.dram_tensor("Wcv", [DIN, DIN], f32, kind="ExternalInput").ap()
    Wdc = nc.dram_tensor("Wdc", [DIN, DIN], f32, kind="ExternalInput").ap()
    bst = nc.dram_tensor("bst", [DIN, 1], f32, kind="ExternalInput").ap()
    bcv = nc.dram_tensor("bcv", [DIN, 1], f32, kind="ExternalInput").ap()
    bdc = nc.dram_tensor("bdc", [DIN, 1], f32, kind="ExternalInput").ap()
    Wint = nc.dram_tensor("Wint", [DIN, K], bf16, kind="ExternalInput").ap()
    bint84 = nc.dram_tensor("bint84", [84, 1], f32, kind="ExternalInput").ap()
    bint20 = nc.dram_tensor("bint20", [20, 1], f32, kind="ExternalInput").ap()
    ntau32 = nc.dram_tensor("ntau32", [32, 4 * H], bf16, kind="ExternalInput").ap()
    SEL32 = nc.dram_tensor("SEL32", [32, 32 * 128], bf16, kind="ExternalInput").ap()
    dtS = nc.dram_tensor("dtS", [1, H], f32, kind="ExternalInput").ap()
    onehT = nc.dram_tensor("onehT", [K, H], f32, kind="ExternalInput").ap()
    pad1 = nc.dram_tensor("pad1", [1, H], f32, kind="ExternalInput").ap()
    out = nc.dram_tensor("out", [1, 2], f32, kind="ExternalOutput").ap()

    # z packing: 12 s-slots per [84,512] psum tile (3 bands at partitions
    # 0/32/64, band-major: slot rem -> (band=rem//4, fc=rem%4)); 8 full
    # tiles + one partial (4 tail slots at band 0, cols 0:512) per block.
    ACOLS = 9 * 512  # 4608 zbuf cols per active block

    with tile.TileContext(nc) as tc, ExitStack() as ctx:
        cpool = ctx.enter_context(tc.tile_pool(name="consts", bufs=1))
        persist = ctx.enter_context(tc.tile_pool(name="persist", bufs=1))

        def cload(ap_dram, shape, dtype, tag):
            t = cpool.tile(shape, dtype, tag=tag)
            nc.sync.dma_start(t[:], ap_dram)
            return t

        L_sb = cload(Ltri, [128, 128], f32, "cL")
        SEL_sb = cload(SELm, [M, 128], f32, "cSEL")
        on1_sb = cload(ones1, [1, 128], f32, "con1")
        on84_sb = cload(ones84, [84, 1], bf16, "con84")
        on20_sb = cload(ones20, [20, 1], f32, "con20")
        on11_sb = cload(one11, [1, 1], f32, "con11")
        Wst_sb = cload(Wst, [DIN, DIN], f32, "cWst")
        Wcv_sb = cload(Wcv, [DIN, DIN], f32, "cWcv")
        Wdc_sb = cload(Wdc, [DIN, DIN], f32, "cWdc")
        bst_sb = cload(bst, [DIN, 1], f32, "cbst")
        bcv_sb = cload(bcv, [DIN, 1], f32, "cbcv")
        bdc_sb = cload(bdc, [DIN, 1], f32, "cbdc")
        Wint_sb = cload(Wint, [DIN, K], bf16, "cWint")
        bint84_sb = cload(bint84, [84, 1], f32, "cbint84")
        bint20_sb = cload(bint20, [20, 1], f32, "cbint20")
        ntau32_sb = cload(ntau32, [32, 4 * H], bf16, "cntau")
        SEL32_sb = cload(SEL32, [32, 32 * 128], bf16, "cSEL32")
        dtS_sb = cload(dtS, [1, H], f32, "cdtS")
        onehT_sb = cload(onehT, [K, H], f32, "conehT")
        pad1_sb = cload(pad1, [1, H], f32, "cpad1")

        embT_all = persist.tile([128, H], f32)      # embed^T per active block
        omT_all = persist.tile([128, H], f32)       # 10*omega, transposed
        cvT_all = persist.tile([128, H], bf16)      # conv^T
        dlT_all = persist.tile([128, H], bf16)      # (start-conv)^T
        zbuf = persist.tile([84, NBLK * ACOLS], bf16)
        z100 = persist.tile([20, H], f32)
        I_all = persist.tile([1, H], f32)
        out_sb = persist.tile([1, 2], f32)
        carN = persist.tile([128, 1], f32)
        carS = persist.tile([M, 1], f32)

        nc.vector.memset(carN[:], 0.0)
        nc.vector.memset(carS[:], 0.0)

        # Track ACT instructions per table-set stage; dependency anchors at
        # stage boundaries keep the ACT stream sorted so only ~3 act-table
        # loads are emitted instead of ~26 (each costs ~2.7us).
        acts_stage = {1: [], 3: [], 4: []}
        act_stage_cur = [1]
        _real_act = nc.scalar.activation

        def _act(*a, **k):
            ins = _real_act(*a, **k)
            acts_stage[act_stage_cur[0]].append(ins)
            return ins

        nc.scalar.activation = _act
        anchor_t = persist.tile([1, 2], f32)

        # ---------- stage 1: attention cumsum (ACT: Exp only) ----------
        s12 = ExitStack()
        p1 = s12.enter_context(tc.tile_pool(name="p1", bufs=3))
        pp1 = s12.enter_context(tc.tile_pool(name="pp1", bufs=2, space="PSUM"))
        for blk in range(12):
            A_t = p1.tile([128, M], f32, tag="A")
            nc.sync.dma_start(A_t[:], A_all[blk * 128:(blk + 1) * 128, :])
            V_t = p1.tile([128, DPHI], f32, tag="V")
            nc.sync.dma_start(V_t[:], V_all[blk * 128:(blk + 1) * 128, :])
            e_t = p1.tile([128, M], f32, tag="e")
            nc.scalar.activation(e_t[:], A_t[:], AF.Exp)
            prod = p1.tile([128, 128], f32, tag="prod")
            for m in range(M):
                nc.vector.tensor_scalar_mul(
                    prod[:, m * DPHI:(m + 1) * DPHI], V_t[:], e_t[:, m:m + 1])
            NT_ps = pp1.tile([128, 128], f32, tag="NT")
            nc.tensor.matmul(NT_ps[:], prod[:], L_sb[:], start=True, stop=True)
            S1_ps = pp1.tile([M, 128], f32, tag="S1")
            nc.tensor.matmul(S1_ps[:], e_t[:], L_sb[:], start=True, stop=True)
            NT_sb = p1.tile([128, 128], f32, tag="NTs")
            nc.vector.tensor_scalar_add(NT_sb[:], NT_ps[:], carN[:])
            S1_sb = p1.tile([M, 128], f32, tag="S1s")
            nc.vector.tensor_scalar_add(S1_sb[:], S1_ps[:], carS[:])
            nc.vector.tensor_copy(carN[:], NT_sb[:, 127:128])
            nc.vector.tensor_copy(carS[:], S1_sb[:, 127:128])
            if blk >= 6:
                a = blk - 6
                r1 = p1.tile([M, 128], f32, tag="r1")
                nc.vector.reciprocal(r1[:], S1_sb[:])
                R_ps = pp1.tile([128, 128], f32, tag="R")
                nc.tensor.matmul(R_ps[:], SEL_sb[:], r1[:], start=True, stop=True)
                nc.vector.tensor_mul(
                    embT_all[:, a * 128:(a + 1) * 128], NT_sb[:], R_ps[:])

        # ---------- stage 2: prologue (ACT: Exp/Ln/Abs/Relu/Square) ----------
        p2 = s12.enter_context(tc.tile_pool(name="p2", bufs=3))
        pp2 = s12.enter_context(tc.tile_pool(name="pp2", bufs=2, space="PSUM"))
        for a in range(NBLK):
            embT = embT_all[:, a * 128:(a + 1) * 128]

            def lin(W_sb, b_sb):
                y_ps = pp2.tile([128, 128], f32, tag="y")
                nc.tensor.matmul(y_ps[:], W_sb[:], embT, start=True, stop=True)
                y_sb = p2.tile([128, 128], f32, tag="ysb")
                nc.vector.tensor_scalar_add(y_sb[:], y_ps[:], b_sb[:])
                return y_sb

            def gelu(y_sb, dst):
                sq = p2.tile([128, 128], f32, tag="sq")
                nc.scalar.activation(sq[:], y_sb[:], AF.Square)
                t_b = p2.tile([128, 128], f32, tag="tb")
                nc.vector.scalar_tensor_tensor(
                    t_b[:], sq[:], 0.044715, y_sb[:], Alu.mult, Alu.mult)
                inner = p2.tile([128, 128], f32, tag="inner")
                nc.vector.tensor_add(inner[:], y_sb[:], t_b[:])
                e2 = p2.tile([128, 128], f32, tag="e2")
                nc.scalar.activation(e2[:], inner[:], AF.Exp, scale=-2.0 * GELU_C)
                den = p2.tile([128, 128], f32, tag="den")
                nc.vector.tensor_scalar_add(den[:], e2[:], 1.0)
                rec = p2.tile([128, 128], f32, tag="rec")
                nc.vector.reciprocal(rec[:], den[:])
                nc.vector.tensor_mul(dst, y_sb[:], rec[:])

            ys = lin(Wst_sb, bst_sb)
            st_bf = p2.tile([128, 128], bf16, tag="stbf")
            gelu(ys, st_bf[:])
            yc = lin(Wcv_sb, bcv_sb)
            gelu(yc, cvT_all[:, a * 128:(a + 1) * 128])
            nc.vector.tensor_sub(
                dlT_all[:, a * 128:(a + 1) * 128], st_bf[:],
                cvT_all[:, a * 128:(a + 1) * 128])
            yd = lin(Wdc_sb, bdc_sb)
            t_abs = p2.tile([128, 128], f32, tag="tabs")
            nc.scalar.activation(t_abs[:], yd[:], AF.Abs, scale=10.0)
            t_e = p2.tile([128, 128], f32, tag="te")
            nc.scalar.activation(t_e[:], t_abs[:], AF.Exp, scale=-1.0)
            t_l = p2.tile([128, 128], f32, tag="tl")
            nc.scalar.activation(t_l[:], t_e[:], AF.Ln, bias=1.0)
            t_r = p2.tile([128, 128], f32, tag="tr")
            nc.scalar.activation(t_r[:], yd[:], AF.Relu, scale=10.0)
            nc.vector.tensor_add(omT_all[:, a * 128:(a + 1) * 128], t_l[:], t_r[:])

        s12.close()
        act_stage_cur[0] = 3
        # ---------- stage 3: MC loop (ACT: Exp/Tanh only) ----------
        # Groups = 4 consecutive p-values (s = 32*g + p), so each p needs ONE
        # broadcast matmul covering all its g-slices (multi-g strided rhs AP).
        # z matmuls batch up to 4 consecutive cell slices into one [20,512]
        # PSUM band write.  s==100 (the dt endpoint) routes to z100.
        GROUPS = []
        for gp in range(16):
            ps = range(2 * gp, 2 * gp + 2)
            svals = []
            spans = []   # (p, [g...]) per broadcast matmul
            for p in ps:
                gs = [g for g in range(4) if 32 * g + p <= 100]
                spans.append((p, gs))
                svals.extend(32 * g + p for g in gs)
            GROUPS.append((spans, svals))
        ntau_v = ntau32_sb[:].rearrange("p (g t) -> p g t", t=H)

        s3 = ExitStack()
        p3 = s3.enter_context(tc.tile_pool(name="p3", bufs=3))
        p3w = s3.enter_context(tc.tile_pool(name="p3w", bufs=2))
        pp3n = s3.enter_context(tc.tile_pool(name="pp3n", bufs=2, space="PSUM"))
        pp3z = s3.enter_context(tc.tile_pool(name="pp3z", bufs=2, space="PSUM"))
        for a in range(NBLK):
            om_w = p3w.tile([128, 1024], f32, tag="om_w")
            dl_w = p3w.tile([128, 1024], bf16, tag="dl_w")
            cv_w = p3w.tile([128, 1024], bf16, tag="cv_w")
            for (wt, srcT) in ((om_w, omT_all), (dl_w, dlT_all), (cv_w, cvT_all)):
                nc.vector.tensor_copy(wt[:, 0:128], srcT[:, a * 128:(a + 1) * 128])
                nc.vector.tensor_copy(wt[:, 128:256], wt[:, 0:128])
                nc.vector.tensor_copy(wt[:, 256:512], wt[:, 0:256])
                nc.vector.tensor_copy(wt[:, 512:1024], wt[:, 0:512])
            mc_pos = 0      # running MC slot index (s != 100) within block a
            z_ps = None
            for spans, svals in GROUPS:
                G = len(svals)
                W_ = G * 128
                NT_ps = pp3n.tile([128, 1024], f32, tag="NT")
                off = 0
                for p, gs in spans:
                    ng = len(gs)
                    nc.tensor.matmul(
                        NT_ps[:, off:off + ng * 128],
                        SEL32_sb[:, p * 128:(p + 1) * 128],
                        ntau_v[:, gs[0]:gs[0] + ng,
                               a * 128:(a + 1) * 128],
                        start=True, stop=True)
                    off += ng * 128
                arg = p3.tile([128, 1024], bf16, tag="arg")
                nc.vector.scalar_tensor_tensor(
                    arg[:, :W_], NT_ps[:, :W_], 1.0, om_w[:, :W_],
                    Alu.mult, Alu.mult)
                E = p3.tile([128, 1024], bf16, tag="E")
                nc.scalar.activation(E[:, :W_], arg[:, :W_], AF.Exp)
                t1 = p3.tile([128, 1024], bf16, tag="t1")
                nc.gpsimd.tensor_mul(t1[:, :W_], E[:, :W_], dl_w[:, :W_])
                t2 = p3.tile([128, 1024], f32, tag="t2")
                nc.gpsimd.tensor_add(t2[:, :W_], t1[:, :W_], cv_w[:, :W_])
                cell = p3.tile([128, 1024], bf16, tag="cell")
                nc.scalar.activation(cell[:, :W_], t2[:, :W_], AF.Tanh)
                # z matmuls: batch runs of consecutive non-100 slices
                j = 0
                while j < G:
                    if svals[j] == 100:
                        z1_ps = pp3z.tile([20, 128], f32, tag="z1")
                        nc.tensor.matmul(
                            z1_ps[:], Wint_sb[:],
                            cell[:, j * 128:(j + 1) * 128],
                            start=True, stop=True)
                        nc.vector.tensor_scalar_add(
                            z100[:, a * 128:(a + 1) * 128], z1_ps[:],
                            bint20_sb[:])
                        j += 1
                        continue
                    # slot bookkeeping: 12 slots per tile, band-major
                    ti, rem = mc_pos // 12, mc_pos % 12
                    band, fc = rem // 4, rem % 4
                    if rem == 0:
                        z_ps = pp3z.tile([84, 512], f32, tag="z")
                        if ti == 8:   # partial tail tile: pre-fill
                            nc.vector.memset(z_ps[:], -20.0)
                    # run length: consecutive non-100 slices, same band, tile
                    run = 1
                    while (j + run < G and svals[j + run] != 100
                           and fc + run < 4 and rem + run < 12):
                        run += 1
                    nc.tensor.matmul(
                        z_ps[band * 32:band * 32 + 20,
                             fc * 128:(fc + run) * 128],
                        Wint_sb[:], cell[:, j * 128:(j + run) * 128],
                        start=True, stop=True)
                    mc_pos += run
                    j += run
                    if mc_pos % 12 == 0 or mc_pos == S:
                        nc.vector.tensor_scalar_add(
                            zbuf[:, a * ACOLS + ti * 512:
                                 a * ACOLS + (ti + 1) * 512],
                            z_ps[:], bint84_sb[:])
        s3.close()
        act_stage_cur[0] = 4
        # ---------- stage 4: softplus + reductions (ACT: Exp/Ln) ----------
        p4 = ctx.enter_context(tc.tile_pool(name="p4", bufs=2))
        pp4 = ctx.enter_context(tc.tile_pool(name="pp4", bufs=1, space="PSUM"))
        for a in range(NBLK):
            for hf in range(2):  # two 2304-col halves per block
                HC = ACOLS // 2
                c0 = a * ACOLS + hf * HC
                spE = p4.tile([84, HC], bf16, tag="spE")
                nc.scalar.activation(spE[:], zbuf[:, c0:c0 + HC], AF.Exp)
                spL = p4.tile([84, HC], bf16, tag="spL")
                nc.scalar.activation(spL[:], spE[:], AF.Ln, bias=1.0)
                SK_ps = pp4.tile([1, HC], f32, tag="SK")
                for i0 in range(0, HC, 512):
                    w = min(512, HC - i0)
                    nc.tensor.matmul(
                        SK_ps[:, i0:i0 + w], on84_sb[:],
                        spL[:, i0:i0 + w], start=True, stop=True)
                Ipart = p4.tile([1, 128], f32, tag="Ipart")
                nc.vector.tensor_reduce(
                    Ipart[:].rearrange("p (x o) -> p x o", o=1),
                    SK_ps[:].rearrange("p (y x) -> p x y", x=128),
                    Axis.X, Alu.add)
                if hf == 0:
                    nc.vector.tensor_copy(I_all[:, a * 128:(a + 1) * 128],
                                          Ipart[:])
                else:
                    nc.vector.tensor_add(I_all[:, a * 128:(a + 1) * 128],
                                         I_all[:, a * 128:(a + 1) * 128],
                                         Ipart[:])
        # log-likelihood path
        spE1 = p4.tile([20, H], f32, tag="spE1")
        nc.scalar.activation(spE1[:], z100[:], AF.Exp)
        spL1 = p4.tile([20, H], f32, tag="spL1")
        nc.scalar.activation(spL1[:], spE1[:], AF.Ln, bias=1.0)
        wsp = p4.tile([20, H], f32, tag="wsp")
        nc.vector.tensor_mul(wsp[:], spL1[:], onehT_sb[:])
        SK1_ps = pp4.tile([1, H], f32, tag="SK1")
        nc.tensor.matmul(SK1_ps[:, 0:512], on20_sb[:], wsp[:, 0:512],
                         start=True, stop=False)
        nc.tensor.matmul(SK1_ps[:, 512:768], on20_sb[:], wsp[:, 512:768],
                         start=True, stop=False)
        nc.tensor.matmul(SK1_ps[:, 0:512], on11_sb[:], pad1_sb[:, 0:512],
                         start=False, stop=True)
        nc.tensor.matmul(SK1_ps[:, 512:768], on11_sb[:], pad1_sb[:, 512:768],
                         start=False, stop=True)
        lg = p4.tile([1, H], f32, tag="lg")
        nc.scalar.activation(lg[:], SK1_ps[:], AF.Ln)
        nc.vector.tensor_reduce(out_sb[:, 0:1], lg[:], Axis.X, Alu.add)
        wI = p4.tile([1, H], f32, tag="wI")
        nc.vector.tensor_mul(wI[:], I_all[:], dtS_sb[:])
        nc.vector.tensor_reduce(out_sb[:, 1:2], wI[:], Axis.X, Alu.add)
        nc.sync.dma_start(out[:], out_sb[:])

        nc.scalar.activation = _real_act
        from concourse.tile import add_dep_helper
        anc1 = nc.scalar.activation(anchor_t[:, 0:1], anchor_t[:, 0:1], AF.Exp)
        anc2 = nc.scalar.activation(anchor_t[:, 1:2], anchor_t[:, 1:2], AF.Exp)
        for a_ in acts_stage[1]:
            add_dep_helper(anc1.ins, a_.ins, reason="act stage 1/2 before 3")
        for a_ in acts_stage[3]:
            add_dep_helper(a_.ins, anc1.ins, reason="act stage 3 after 1/2")
            add_dep_helper(anc2.ins, a_.ins, reason="act stage 3 before 4")
        for a_ in acts_stage[4]:
            add_dep_helper(a_.ins, anc2.ins, reason="act stage 4 after 3")

    nc.finalize()
    return nc


def _host_prep(values, preattention, mask, seq_times, taus_u, seq_types,
               W_start, b_start, W_conv, b_conv, W_dec, b_dec, W_int, b_int):
    f32 = np.float32
    bf16 = ml_dtypes.bfloat16
    values = np.asarray(values, f32)
    preattention = np.asarray(preattention, f32)
    mask = np.asarray(mask, f32)
    seq_times = np.asarray(seq_times, f32)
    taus_u = np.asarray(taus_u, f32)
    seq_types = np.asarray(seq_types)

    dt = (seq_times[:, 1:] - seq_times[:, :-1]) * mask[:, 1:]        # [B,T]
    u_aug = np.concatenate(
        [taus_u[:, :, 0, :], np.ones((B, T, 1), f32)], axis=-1)      # [B,T,101]
    ntau_full = (-dt[:, :, None] * u_aug / 10.0).astype(f32)         # [B,T,101]
    k_idx = seq_types[:, 1:].astype(np.int64) - 1
    oh = ((k_idx[:, :, None] == np.arange(K)[None, None, :])
          & (k_idx[:, :, None] >= 0) & (k_idx[:, :, None] < K)).astype(f32)

    sel32 = np.zeros((32, 32 * 128), bf16)
    for j in range(32):
        sel32[j, j * 128:(j + 1) * 128] = 1.0
    shared = dict(
        Ltri=np.triu(np.ones((128, 128), f32)),
        SELm=np.repeat(np.eye(M, dtype=f32), DPHI, axis=1),
        SEL32=sel32,
        ones1=np.ones((1, 128), f32),
        ones84=np.where(np.arange(84)[:, None] % 32 < 20, 1.0, 0.0).astype(bf16),
        ones20=np.ones((20, 1), f32),
        one11=np.ones((1, 1), f32),
        Wst=W_start.astype(f32), Wcv=W_conv.astype(f32), Wdc=W_dec.astype(f32),
        bst=b_start.astype(f32).reshape(DIN, 1),
        bcv=b_conv.astype(f32).reshape(DIN, 1),
        bdc=b_dec.astype(f32).reshape(DIN, 1),
        Wint=W_int.astype(bf16),
        bint84=np.concatenate(
            [np.pad(b_int.astype(f32), (0, 12)) for _ in range(3)])[:84]
        .reshape(84, 1),
        bint20=b_int.astype(f32).reshape(20, 1),
    )

    in_maps = []
    for core in range(8):
        b, half = core // 2, core % 2
        t0 = half * H
        if half == 0:
            A_pref = np.full((H, M), -60.0, f32)
            V_pref = np.zeros((H, DPHI), f32)
        else:
            A_pref = preattention[b, :H]
            V_pref = values[b, :H]
        A_act = preattention[b, t0:t0 + H]
        V_act = values[b, t0:t0 + H]

        ntau_c = np.zeros((NS, H), f32)
        dtS_c = np.zeros((1, H), f32)
        oh_c = np.zeros((K, H), f32)
        pad1_c = np.zeros((1, H), f32)
        nvalid = min(T - t0, H)
        ntau_c[:, :nvalid] = ntau_full[b, t0:t0 + nvalid].T
        dtS_c[0, :nvalid] = dt[b, t0:t0 + nvalid] / float(S)
        oh_c[:, :nvalid] = oh[b, t0:t0 + nvalid].T
        pad1_c[0, nvalid:] = 1.0
        ntau32_c = np.zeros((32, 4 * H), bf16)
        for s in range(NS):
            ntau32_c[s % 32, (s // 32) * H:(s // 32 + 1) * H] = ntau_c[s]

        m = dict(shared)
        m.update(
            A_all=np.concatenate([A_pref, A_act], 0),
            V_all=np.concatenate([V_pref, V_act], 0),
            ntau32=ntau32_c, dtS=dtS_c, onehT=oh_c, pad1=pad1_c,
        )
        in_maps.append(m)
    return in_maps


def kernel(**inputs) -> np.ndarray:
    from concourse.bass_utils import run_bass_kernel_spmd

    if "nc" not in _CACHE:
        _CACHE["nc"] = _build_nc()
    nc = _CACHE["nc"]
    in_maps = _host_prep(**inputs)
    trace = bool(int(os.environ.get("KTRACE", "0")))
    res = run_bass_kernel_spmd(nc, in_maps, core_ids=list(range(8)), trace=trace)
    if trace:
        _CACHE["last_result"] = res
        print("HW exec time:", res.exec_time_ns, "ns")
    outs = np.stack([np.asarray(r["out"]).reshape(2) for r in res.results])
    full = outs.reshape(B, 2, 2).sum(axis=1)   # sum the two halves per batch
    return full.astype(np.float32)
